# revision 4
# baseline (speedup 1.0000x reference)
"""ARMA GNN (2-layer, K=2 stacks) Trainium2 kernel.

Strategy (8-core SPMD, node-sharded):
  * norm folding: norm[e] = dinv[row]*dinv[col].  Gather tables hold
    dinv[row]*x[row] @ Wcat (both stacks concatenated on the feature axis),
    dinv[col] is applied per target window at the epilogue.
  * Edges are sharded by target (col) across cores; each core owns
    SHARD consecutive nodes = W windows of 128 target slots.
  * Per 128-edge block: gather source rows with dma_gather (SWDGE custom
    DMA), build a 0/1 selection matrix S[e, slot] = (iota == slot[e]) on
    DVE, and matmul S.T @ M on PE, accumulating windows in PSUM.
  * Sources are split in two table halves (row index must fit int16).
  * Between layers: one AllGather of the [64, SHARD] h1^T shards.
  * All programs identical across cores (block counts padded to the
    max over cores per window); per-core behavior comes from input data.

kernel(**inputs) takes the FULL problem inputs and returns the FULL output.
"""

import sys

sys.path.insert(0, "/opt/trn_rl_repo")

from contextlib import ExitStack

import numpy as np

P = 128


# --------------------------------------------------------------------------
# configuration
# --------------------------------------------------------------------------
class Cfg:
    def __init__(self, N, NC, SHARD, B0, ZPAD=256, WB=2, G=8, SUPER=4,
                 FIN=128, HID=64, FOUT=64, K=2):
        self.N, self.NC, self.SHARD, self.B0, self.ZPAD = N, NC, SHARD, B0, ZPAD
        self.WB, self.G, self.SUPER = WB, G, SUPER
        self.FIN, self.HID, self.FOUT, self.K = FIN, HID, FOUT, K
        self.NSTAR = NC * SHARD
        self.W = SHARD // P
        self.ROWS = self.NSTAR + ZPAD
        self.HALFA = B0 + ZPAD
        self.HALFB = self.NSTAR - B0
        self.NCHUNK = self.ROWS // P
        self.ACH = self.HALFA // P      # chunks in half A
        self.BCH = self.HALFB // P
        assert B0 % P == 0 and SHARD % P == 0 and ZPAD % P == 0
        assert self.HALFA < 32768 and self.HALFB < 32768
        assert N > B0 and N < self.NSTAR
        assert K * HID == 128 and K * FOUT == 128 and FIN == 128
        self.ZLOCA = B0                       # zero row (local) in half A
        self.ZLOCB = (N + ZPAD) - self.HALFA  # zero row (local) in half B
        # gather batches of WB windows
        self.batches = [tuple(range(b, min(b + WB, self.W)))
                        for b in range(0, self.W, WB)]
        # superbatches of SUPER batches (idx staging granularity)
        self.sbatches = [self.batches[i:i + SUPER]
                         for i in range(0, len(self.batches), SUPER)]


REAL = dict(N=50000, NC=8, SHARD=6272, B0=24960)


# --------------------------------------------------------------------------
# host preprocessing
# --------------------------------------------------------------------------
def _preprocess(c: Cfg, x, edge_index, init_w1, root_w1, b1, init_w2, root_w2, b2):
    N, NC, SHARD = c.N, c.NC, c.SHARD
    row = np.asarray(edge_index[0]).astype(np.int64)
    col = np.asarray(edge_index[1]).astype(np.int64)
    x = np.asarray(x, dtype=np.float32)

    deg = np.bincount(col, minlength=N).astype(np.float64)
    dinv = np.where(deg > 0, deg ** -0.5, 0.0).astype(np.float32)
    dinv_full = np.zeros(c.NSTAR, np.float32)
    dinv_full[:N] = dinv

    srow = row + (row >= c.B0) * c.ZPAD  # table row of each source

    # ---- per-core edge buckets -------------------------------------------
    percore = []
    counts = np.zeros((NC, c.W, 2), np.int64)
    for cc in range(NC):
        base = cc * SHARD
        m = (col >= base) & (col < base + SHARD)
        ec = (col[m] - base).astype(np.int64)
        es = srow[m]
        half = (es >= c.HALFA).astype(np.int64)
        key = (ec >> 7) * 2 + half
        order = np.argsort(key, kind="stable")
        ec, es, key = ec[order], es[order], key[order]
        bounds = np.searchsorted(key, np.arange(2 * c.W + 1))
        percore.append((ec, es, bounds))
        for w in range(c.W):
            counts[cc, w, 0] = bounds[2 * w + 1] - bounds[2 * w]
            counts[cc, w, 1] = bounds[2 * w + 2] - bounds[2 * w + 1]

    # shared, per-window padded block counts
    NBA = [max(1, int(-(-counts[:, w, 0].max() // P))) for w in range(c.W)]
    NBB = [max(1, int(-(-counts[:, w, 1].max() // P))) for w in range(c.W)]

    # stream assembly (order: per batch -> per window A blocks; then B)
    def build_stream(cc, half_id, NB_list):
        ec, es, bounds = percore[cc]
        toks, slots = [], []
        zloc = c.ZLOCA if half_id == 0 else c.ZLOCB
        for batch in c.batches:
            for w in batch:
                lo, hi = bounds[2 * w + half_id], bounds[2 * w + half_id + 1]
                k = hi - lo
                n = NB_list[w] * P
                t = np.full(n, zloc, np.int64)
                s = np.zeros(n, np.int64)
                t[:k] = es[lo:hi] - (c.HALFA if half_id else 0)
                s[:k] = ec[lo:hi] & 127
                toks.append(t)
                slots.append(s)
        toks = np.concatenate(toks)
        slots = np.concatenate(slots)
        L = len(toks)
        idxw = np.tile(toks.reshape(L // 16, 16).T.astype(np.int16), (8, 1))
        colf = np.ascontiguousarray(slots.reshape(L // P, P).T.astype(np.float32))
        return np.ascontiguousarray(idxw), colf

    # ---- dense tensors ----------------------------------------------------
    xrow = np.zeros((c.ROWS, c.FIN), np.float32)
    xrow[:c.B0] = x[:c.B0]
    xrow[c.HALFA:c.HALFA + (N - c.B0)] = x[c.B0:N]
    xT = np.ascontiguousarray(xrow.T)

    dinvrow = np.zeros(c.ROWS, np.float32)
    dinvrow[:c.B0] = dinv_full[:c.B0]
    dinvrow[c.HALFA:] = dinv_full[c.B0:]
    dinvr = np.ascontiguousarray(dinvrow.reshape(c.NCHUNK, P).T)

    xpad = np.zeros((c.NSTAR, c.FIN), np.float32)
    xpad[:N] = x

    def cat2(w):  # [K, fi, fo] -> [fi, K*fo]
        w = np.asarray(w, dtype=np.float32)
        return np.ascontiguousarray(np.concatenate([w[0], w[1]], axis=1))

    w1cat = cat2(init_w1)
    rw1c = 0.5 * cat2(root_w1)
    w2cat = cat2(init_w2)
    rw2c = 0.5 * cat2(root_w2)
    b1 = np.asarray(b1, dtype=np.float32)
    b2 = np.asarray(b2, dtype=np.float32)
    b1b = np.ascontiguousarray(np.tile(0.5 * np.concatenate([b1[0], b1[1]]), (P, 1)))
    b2b = np.ascontiguousarray(np.tile(0.5 * np.concatenate([b2[0], b2[1]]), (P, 1)))

    in_maps = []
    for cc in range(NC):
        base = cc * SHARD
        idxA, colfA = build_stream(cc, 0, NBA)
        idxB, colfB = build_stream(cc, 1, NBB)
        dinvo = 0.5 * dinv_full[base:base + SHARD].reshape(c.W, P).T
        in_maps.append({
            "xT": xT,
            "xTow": np.ascontiguousarray(xpad[base:base + SHARD].T),
            "w1cat": w1cat, "rw1c": rw1c, "w2cat": w2cat, "rw2c": rw2c,
            "b1b": b1b, "b2b": b2b,
            "dinvr": dinvr,
            "dinvo": np.ascontiguousarray(dinvo.astype(np.float32)),
            "idxA": idxA, "idxB": idxB,
            "colfA": colfA, "colfB": colfB,
        })
    return in_maps, NBA, NBB


# --------------------------------------------------------------------------
# device program
# --------------------------------------------------------------------------
def _build_program(c: Cfg, NBA, NBB):
    import concourse.tile as tile
    from concourse import bacc, mybir
    from concourse.masks import make_identity

    f32 = mybir.dt.float32
    i16 = mybir.dt.int16
    AL = mybir.AluOpType
    AF = mybir.ActivationFunctionType

    NBAtot, NBBtot = sum(NBA), sum(NBB)
    LA, LB = NBAtot * P, NBBtot * P

    nc = bacc.Bacc("TRN2", target_bir_lowering=False, debug=False,
                   num_devices=c.NC)

    def din(name, shape, dt=f32):
        return nc.dram_tensor(name, shape, dt, kind="ExternalInput")

    xT = din("xT", [P, c.ROWS])
    xTow = din("xTow", [P, c.SHARD])
    w1cat = din("w1cat", [P, 128]); rw1c = din("rw1c", [P, 128])
    w2cat = din("w2cat", [64, 128]); rw2c = din("rw2c", [64, 128])
    b1b = din("b1b", [P, 128]); b2b = din("b2b", [P, 128])
    dinvr = din("dinvr", [P, c.NCHUNK])
    dinvo = din("dinvo", [P, c.W])
    idxA = din("idxA", [P, LA // 16], i16)
    idxB = din("idxB", [P, LB // 16], i16)
    colfA = din("colfA", [P, NBAtot])
    colfB = din("colfB", [P, NBBtot])
    yt = nc.dram_tensor("yt", [64, c.SHARD], f32, kind="ExternalOutput")

    t1A = nc.dram_tensor("t1A", [c.HALFA, 128], f32)
    t1B = nc.dram_tensor("t1B", [c.HALFB, 128], f32)
    t2A = nc.dram_tensor("t2A", [c.HALFA, 128], f32)
    t2B = nc.dram_tensor("t2B", [c.HALFB, 128], f32)
    ccin = nc.dram_tensor("ccin", [64, c.SHARD], f32)
    ccout = nc.dram_tensor("ccout", [c.NC, 64, c.SHARD], f32)

    # table2 chunk -> (shard, local node chunk) map; zero chunks -> None
    chunk_src = [None] * c.NCHUNK
    for s in range(c.NC):
        for j in range(c.W):
            node0 = s * c.SHARD + j * P
            r0 = node0 + (c.ZPAD if node0 >= c.B0 else 0)
            chunk_src[r0 // P] = (s, j)
    runs = []
    i = 0
    while i < c.NCHUNK:
        if chunk_src[i] is None:
            j = i
            while j < c.NCHUNK and chunk_src[j] is None:
                j += 1
            runs.append(("zero", i, j - i, 0))
            i = j
        else:
            s0, l0 = chunk_src[i]
            j = i
            while (j < c.NCHUNK and chunk_src[j] is not None
                   and chunk_src[j][0] == s0
                   and chunk_src[j][1] == l0 + (j - i) and (j - i) < 12):
                j += 1
            runs.append((s0, i, j - i, l0))
            i = j

    with tile.TileContext(nc) as tc, ExitStack() as ctx:
        cpool = ctx.enter_context(tc.tile_pool(name="consts", bufs=1))
        xtp = ctx.enter_context(tc.tile_pool(name="xtp", bufs=2))
        stg = ctx.enter_context(tc.tile_pool(name="stg", bufs=2))
        gth = ctx.enter_context(tc.tile_pool(name="gth", bufs=3))
        sgp = ctx.enter_context(tc.tile_pool(name="sgp", bufs=8))
        idxp = ctx.enter_context(tc.tile_pool(name="idxp", bufs=2))
        epi = ctx.enter_context(tc.tile_pool(name="epi", bufs=3))
        big = ctx.enter_context(tc.tile_pool(name="big", bufs=1))
        shp = ctx.enter_context(tc.tile_pool(name="shp", bufs=2))
        psx = ctx.enter_context(tc.tile_pool(name="psx", bufs=2, space="PSUM"))
        psw = ctx.enter_context(tc.tile_pool(name="psw", bufs=3, space="PSUM"))

        # ---- constants ----
        ident = cpool.tile([P, P], f32, tag="ident")
        make_identity(nc, ident[:])
        iota_i = cpool.tile([P, c.G * P], mybir.dt.int32, tag="iotai")
        nc.gpsimd.iota(iota_i[:], pattern=[[0, c.G], [1, P]], base=0,
                       channel_multiplier=0)
        iota_f = cpool.tile([P, c.G * P], f32, tag="iotaf")
        nc.vector.tensor_copy(iota_f[:], iota_i[:])
        zero64 = cpool.tile([64, 128], f32, tag="zero64")
        nc.vector.memset(zero64[:], 0.0)

        def load_const(dram, shape, tag, dt=f32):
            t = cpool.tile(shape, dt, tag=tag)
            nc.sync.dma_start(t[:], dram[:, :])
            return t

        w1_s = load_const(w1cat, [P, 128], "w1")
        rw1_s = load_const(rw1c, [P, 128], "rw1")
        w2_s = load_const(w2cat, [64, 128], "w2")
        rw2_s = load_const(rw2c, [64, 128], "rw2")
        b1_s = load_const(b1b, [P, 128], "b1")
        b2_s = load_const(b2b, [P, 128], "b2")
        dinvr_s = load_const(dinvr, [P, c.NCHUNK], "dinvr")
        dinvo_s = load_const(dinvo, [P, c.W], "dinvo")
        colfA_s = load_const(colfA, [P, NBAtot], "colfA")
        colfB_s = load_const(colfB, [P, NBBtot], "colfB")

        def half_base(t):
            return 0 if t in (t1A, t2A) else c.ACH

        # ---- staged table builder: chunk_list = [(tgt, rc, lhsT_ap)] ----
        def build_table(chunk_list, rhs_tile):
            i = 0
            while i < len(chunk_list):
                grp = chunk_list[i:i + 8]
                tgt = grp[0][0]
                grp = [g for g in grp if g[0] is tgt]
                st = stg.tile([P, 8 * 128], f32, tag="stage")
                for j, (_, rc, lap) in enumerate(grp):
                    ps = psx.tile([P, 128], f32)
                    nc.tensor.matmul(out=ps[:], lhsT=lap, rhs=rhs_tile[:],
                                     start=True, stop=True)
                    nc.vector.tensor_scalar(
                        out=st[:, j * 128:(j + 1) * 128], in0=ps[:],
                        scalar1=dinvr_s[:, rc:rc + 1], scalar2=None,
                        op0=AL.mult)
                r0 = grp[0][1] - half_base(tgt)
                n = len(grp)
                out_ap = tgt[r0 * P:(r0 + n) * P, :] \
                    .rearrange("(k p) f -> p k f", p=P)
                nc.sync.dma_start(out_ap, st[:, :n * 128])
                i += n

        # ---- layer-1 tables (A half then B half) ----
        i = 0
        chunk_list1 = []
        while i < c.NCHUNK:
            n = min(8, c.NCHUNK - i)
            if i < c.ACH < i + n:
                n = c.ACH - i
            xp = xtp.tile([P, 8 * 128], f32, tag="xtp")
            nc.sync.dma_start(xp[:, :n * 128], xT[:, i * P:(i + n) * P])
            for j in range(n):
                rc = i + j
                tgt = t1A if rc < c.ACH else t1B
                chunk_list1.append((tgt, rc, xp[:, j * 128:(j + 1) * 128]))
            i += n
        build_table(chunk_list1, w1_s)

        # ---- root1 (+ bias, pre-halved) ----
        root1 = big.tile([P, c.SHARD], f32, tag="root")
        i = 0
        while i < c.W:
            n = min(8, c.W - i)
            xp = xtp.tile([P, 8 * 128], f32, tag="xtp")
            nc.sync.dma_start(xp[:, :n * 128], xTow[:, i * P:(i + n) * P])
            for j in range(n):
                ps = psx.tile([P, 128], f32)
                nc.tensor.matmul(out=ps[:], lhsT=xp[:, j * 128:(j + 1) * 128],
                                 rhs=rw1_s[:], start=True, stop=True)
                nc.vector.tensor_tensor(
                    out=root1[:, (i + j) * 128:(i + j + 1) * 128],
                    in0=ps[:], in1=b1_s[:], op=AL.add)
            i += n

        # ---- gather/aggregate layer ----
        def layer(tabA, tabB, root_t, out_t):
            blkA = blkB = 0       # global block counters
            tokA = tokB = 0       # global token counters
            for sb in c.sbatches:
                sbA = sum(NBA[w] for b in sb for w in b) * P
                sbB = sum(NBB[w] for b in sb for w in b) * P
                ixA = idxp.tile([P, sbA // 16], i16, tag="ixA")
                nc.sync.dma_start(ixA[:], idxA[:, tokA // 16:(tokA + sbA) // 16])
                ixB = idxp.tile([P, sbB // 16], i16, tag="ixB")
                nc.sync.dma_start(ixB[:], idxB[:, tokB // 16:(tokB + sbB) // 16])
                lA = lB = 0       # local token offset within superbatch
                for batch in sb:
                    nA = sum(NBA[w] for w in batch)
                    nB = sum(NBB[w] for w in batch)
                    gA = gth.tile([P, nA * 128], f32, tag="gath")
                    nc.gpsimd.dma_gather(
                        out_ap=gA[:].rearrange("p (b f) -> p b f", f=128),
                        in_ap=tabA[:, :],
                        idxs_ap=ixA[:, lA // 16:(lA + nA * P) // 16],
                        num_idxs=nA * P, num_idxs_reg=nA * P, elem_size=128,
                        single_packet=False)
                    gB = gth.tile([P, nB * 128], f32, tag="gath")
                    nc.gpsimd.dma_gather(
                        out_ap=gB[:].rearrange("p (b f) -> p b f", f=128),
                        in_ap=tabB[:, :],
                        idxs_ap=ixB[:, lB // 16:(lB + nB * P) // 16],
                        num_idxs=nB * P, num_idxs_reg=nB * P, elem_size=128,
                        single_packet=False)
                    # S tiles for this batch
                    sA, sB_ = [], []
                    for g0 in range(0, nA, c.G):
                        gl = min(c.G, nA - g0)
                        s_t = sgp.tile([P, c.G * 128], f32, tag="sg")
                        nc.vector.tensor_tensor(
                            out=s_t[:, :gl * 128], in0=iota_f[:, :gl * 128],
                            in1=colfA_s[:, blkA + g0:blkA + g0 + gl]
                                .to_broadcast([P, gl, 128]),
                            op=AL.is_equal)
                        sA.append(s_t)
                    for g0 in range(0, nB, c.G):
                        gl = min(c.G, nB - g0)
                        s_t = sgp.tile([P, c.G * 128], f32, tag="sg")
                        nc.vector.tensor_tensor(
                            out=s_t[:, :gl * 128], in0=iota_f[:, :gl * 128],
                            in1=colfB_s[:, blkB + g0:blkB + g0 + gl]
                                .to_broadcast([P, gl, 128]),
                            op=AL.is_equal)
                        sB_.append(s_t)
                    # windows
                    oA = oB = 0
                    for w in batch:
                        pw = psw.tile([P, 128], f32)
                        nmm = NBA[w] + NBB[w]
                        k = 0
                        for j in range(NBA[w]):
                            b = oA + j
                            nc.tensor.matmul(
                                out=pw[:],
                                lhsT=sA[b // c.G][:, (b % c.G) * 128:(b % c.G + 1) * 128],
                                rhs=gA[:, b * 128:(b + 1) * 128],
                                start=(k == 0), stop=(k == nmm - 1))
                            k += 1
                        for j in range(NBB[w]):
                            b = oB + j
                            nc.tensor.matmul(
                                out=pw[:],
                                lhsT=sB_[b // c.G][:, (b % c.G) * 128:(b % c.G + 1) * 128],
                                rhs=gB[:, b * 128:(b + 1) * 128],
                                start=(k == 0), stop=(k == nmm - 1))
                            k += 1
                        oA += NBA[w]; oB += NBB[w]
                        # epilogue: relu(psum*dinv + root), sum stacks, transpose
                        t2 = epi.tile([P, 128], f32, tag="t2")
                        nc.vector.scalar_tensor_tensor(
                            out=t2[:], in0=pw[:], scalar=dinvo_s[:, w:w + 1],
                            in1=root_t[:, w * 128:(w + 1) * 128],
                            op0=AL.mult, op1=AL.add)
                        t3 = epi.tile([P, 128], f32, tag="t3")
                        nc.scalar.activation(t3[:], t2[:], AF.Relu)
                        t4 = epi.tile([P, 64], f32, tag="t4")
                        nc.vector.tensor_tensor(out=t4[:], in0=t3[:, :64],
                                                in1=t3[:, 64:], op=AL.add)
                        pt = psx.tile([64, 128], f32)
                        nc.tensor.transpose(out=pt[:], in_=t4[:],
                                            identity=ident[:])
                        nc.scalar.copy(out_t[:, w * 128:(w + 1) * 128], pt[:])
                    blkA += nA; blkB += nB
                    lA += nA * P; lB += nB * P
                tokA += sbA; tokB += sbB

        h1t = big.tile([64, c.SHARD], f32, tag="ht")
        layer(t1A, t1B, root1, h1t)

        # ---- allgather h1^T ----
        nc.sync.dma_start(ccin[:, :], h1t[:])
        nc.gpsimd.collective_compute(
            "AllGather", AL.bypass,
            replica_groups=[list(range(c.NC))],
            ins=[ccin.ap().opt()], outs=[ccout.ap().opt()])

        # ---- root2 from local h1t ----
        root2 = big.tile([P, c.SHARD], f32, tag="root")
        for j in range(c.W):
            ps = psx.tile([P, 128], f32)
            nc.tensor.matmul(out=ps[:], lhsT=h1t[:, j * 128:(j + 1) * 128],
                             rhs=rw2_s[:], start=True, stop=True)
            nc.vector.tensor_tensor(out=root2[:, j * 128:(j + 1) * 128],
                                    in0=ps[:], in1=b2_s[:], op=AL.add)

        # ---- table2 from allgathered h1 ----
        chunk_list2 = []
        for run in runs:
            kind, rc0, n, l0 = run
            if kind == "zero":
                for j in range(n):
                    rc = rc0 + j
                    tgt = t2A if rc < c.ACH else t2B
                    chunk_list2.append((tgt, rc, zero64[:]))
            else:
                pc = shp.tile([64, 12 * 128], f32, tag="h1pc")
                nc.sync.dma_start(pc[:, :n * 128],
                                  ccout[kind, :, l0 * P:(l0 + n) * P])
                for j in range(n):
                    rc = rc0 + j
                    tgt = t2A if rc < c.ACH else t2B
                    chunk_list2.append((tgt, rc, pc[:, j * 128:(j + 1) * 128]))
        build_table(chunk_list2, w2_s)

        yts = big.tile([64, c.SHARD], f32, tag="ht")
        layer(t2A, t2B, root2, yts)
        nc.sync.dma_start(yt[:, :], yts[:])

    nc.compile()
    return nc


# --------------------------------------------------------------------------
# entry points
# --------------------------------------------------------------------------
_cache = {}


def prepare(inputs, cfg_kw=None):
    """Preprocess + build (cached by config + padded block structure)."""
    c = Cfg(**(cfg_kw or REAL))
    in_maps, NBA, NBB = _preprocess(c, **inputs)
    key = (tuple(sorted((cfg_kw or REAL).items())), tuple(NBA), tuple(NBB))
    if key not in _cache:
        _cache[key] = _build_program(c, NBA, NBB)
    return c, _cache[key], in_maps


def kernel(x, edge_index, init_w1, root_w1, b1, init_w2, root_w2, b2,
           _trace=False, _cfg=None):
    from concourse import bass_utils
    inputs = dict(x=np.asarray(x), edge_index=np.asarray(edge_index),
                  init_w1=np.asarray(init_w1), root_w1=np.asarray(root_w1),
                  b1=np.asarray(b1), init_w2=np.asarray(init_w2),
                  root_w2=np.asarray(root_w2), b2=np.asarray(b2))
    c, nc, in_maps = prepare(inputs, _cfg)
    res = bass_utils.run_bass_kernel_spmd(
        nc, in_maps, core_ids=list(range(c.NC)), trace=_trace)
    out = np.concatenate([res.results[cc]["yt"].T for cc in range(c.NC)],
                         axis=0)[:c.N]
    if _trace:
        kernel._last = res
    return np.ascontiguousarray(out.astype(np.float32))


# revision 5
# speedup vs baseline: 2.1537x; 2.1537x over previous
"""ARMA GNN (2-layer, K=2 stacks) Trainium2 kernel.

Strategy (8-core SPMD, node-sharded):
  * norm folding: norm[e] = dinv[row]*dinv[col].  Gather tables hold
    dinv[row]*x[row] @ Wcat (both stacks concatenated on the feature axis,
    fp16), dinv[col] is applied per target window at the epilogue.
  * Edges are sharded by target (col) across cores; each core owns
    SHARD consecutive nodes = W windows of 128 target slots.
  * Per 128-edge block: gather source rows with dma_gather (SWDGE custom
    DMA, 4 queues round-robin so all four Q7 core pairs generate
    descriptors concurrently), build a 0/1 selection matrix
    S[e, slot] = (iota == slot[e]) on DVE (fp16), and matmul S.T @ M on
    PE (fp16 with FWL), accumulating windows in f32 PSUM.
  * Sources are split in two table halves (row index must fit int16).
  * Between layers: one AllGather of the [64, SHARD] fp16 h1^T shards.
  * All programs identical across cores (block counts padded to the
    max over cores per window); per-core behavior comes from input data.

kernel(**inputs) takes the FULL problem inputs and returns the FULL output.
"""

import sys

sys.path.insert(0, "/opt/trn_rl_repo")

from contextlib import ExitStack

import numpy as np

P = 128


# --------------------------------------------------------------------------
# configuration
# --------------------------------------------------------------------------
class Cfg:
    def __init__(self, N, NC, SHARD, B0, ZPAD=256, WB=2, G=8, SUPER=4,
                 FIN=128, HID=64, FOUT=64, K=2):
        self.N, self.NC, self.SHARD, self.B0, self.ZPAD = N, NC, SHARD, B0, ZPAD
        self.WB, self.G, self.SUPER = WB, G, SUPER
        self.FIN, self.HID, self.FOUT, self.K = FIN, HID, FOUT, K
        self.NSTAR = NC * SHARD
        self.W = SHARD // P
        self.ROWS = self.NSTAR + ZPAD
        self.HALFA = B0 + ZPAD
        self.HALFB = self.NSTAR - B0
        self.NCHUNK = self.ROWS // P
        self.ACH = self.HALFA // P      # chunks in half A
        self.BCH = self.HALFB // P
        assert B0 % P == 0 and SHARD % P == 0 and ZPAD % P == 0
        assert self.HALFA < 32768 and self.HALFB < 32768
        assert N > B0 and N < self.NSTAR
        assert K * HID == 128 and K * FOUT == 128 and FIN == 128
        self.ZLOCA = B0                       # zero row (local) in half A
        self.ZLOCB = (N + ZPAD) - self.HALFA  # zero row (local) in half B
        self.batches = [tuple(range(b, min(b + WB, self.W)))
                        for b in range(0, self.W, WB)]
        self.sbatches = [self.batches[i:i + SUPER]
                         for i in range(0, len(self.batches), SUPER)]


REAL = dict(N=50000, NC=8, SHARD=6272, B0=24960)


# --------------------------------------------------------------------------
# host preprocessing
# --------------------------------------------------------------------------
def _preprocess(c: Cfg, x, edge_index, init_w1, root_w1, b1, init_w2, root_w2, b2):
    N, NC, SHARD = c.N, c.NC, c.SHARD
    row = np.asarray(edge_index[0]).astype(np.int64)
    col = np.asarray(edge_index[1]).astype(np.int64)
    x = np.asarray(x, dtype=np.float32)

    deg = np.bincount(col, minlength=N).astype(np.float64)
    dinv = np.where(deg > 0, deg ** -0.5, 0.0).astype(np.float32)
    dinv_full = np.zeros(c.NSTAR, np.float32)
    dinv_full[:N] = dinv

    srow = row + (row >= c.B0) * c.ZPAD  # table row of each source

    percore = []
    counts = np.zeros((NC, c.W, 2), np.int64)
    for cc in range(NC):
        base = cc * SHARD
        m = (col >= base) & (col < base + SHARD)
        ec = (col[m] - base).astype(np.int64)
        es = srow[m]
        half = (es >= c.HALFA).astype(np.int64)
        key = (ec >> 7) * 2 + half
        order = np.argsort(key, kind="stable")
        ec, es, key = ec[order], es[order], key[order]
        bounds = np.searchsorted(key, np.arange(2 * c.W + 1))
        percore.append((ec, es, bounds))
        for w in range(c.W):
            counts[cc, w, 0] = bounds[2 * w + 1] - bounds[2 * w]
            counts[cc, w, 1] = bounds[2 * w + 2] - bounds[2 * w + 1]

    NBA = [max(1, int(-(-counts[:, w, 0].max() // P))) for w in range(c.W)]
    NBB = [max(1, int(-(-counts[:, w, 1].max() // P))) for w in range(c.W)]

    def build_stream(cc, half_id, NB_list):
        ec, es, bounds = percore[cc]
        toks, slots = [], []
        zloc = c.ZLOCA if half_id == 0 else c.ZLOCB
        for batch in c.batches:
            for w in batch:
                lo, hi = bounds[2 * w + half_id], bounds[2 * w + half_id + 1]
                k = hi - lo
                n = NB_list[w] * P
                t = np.full(n, zloc, np.int64)
                s = np.zeros(n, np.int64)
                t[:k] = es[lo:hi] - (c.HALFA if half_id else 0)
                s[:k] = ec[lo:hi] & 127
                toks.append(t)
                slots.append(s)
        toks = np.concatenate(toks)
        slots = np.concatenate(slots)
        L = len(toks)
        idxw = np.tile(toks.reshape(L // 16, 16).T.astype(np.int16), (8, 1))
        colf = np.ascontiguousarray(slots.reshape(L // P, P).T.astype(np.float16))
        return np.ascontiguousarray(idxw), colf

    xrow = np.zeros((c.ROWS, c.FIN), np.float32)
    xrow[:c.B0] = x[:c.B0]
    xrow[c.HALFA:c.HALFA + (N - c.B0)] = x[c.B0:N]
    xT = np.ascontiguousarray(xrow.T.astype(np.float16))

    dinvrow = np.zeros(c.ROWS, np.float32)
    dinvrow[:c.B0] = dinv_full[:c.B0]
    dinvrow[c.HALFA:] = dinv_full[c.B0:]
    dinvr = np.ascontiguousarray(dinvrow.reshape(c.NCHUNK, P).T)

    xpad = np.zeros((c.NSTAR, c.FIN), np.float32)
    xpad[:N] = x

    def cat2(w):  # [K, fi, fo] -> [fi, K*fo] fp16
        w = np.asarray(w, dtype=np.float32)
        return np.ascontiguousarray(
            np.concatenate([w[0], w[1]], axis=1).astype(np.float16))

    w1cat = cat2(init_w1)
    rw1c = cat2(0.5 * np.asarray(root_w1, np.float32))
    w2cat = cat2(init_w2)
    rw2c = cat2(0.5 * np.asarray(root_w2, np.float32))
    b1 = np.asarray(b1, dtype=np.float32)
    b2 = np.asarray(b2, dtype=np.float32)
    b1b = np.ascontiguousarray(np.tile(0.5 * np.concatenate([b1[0], b1[1]]), (P, 1)))
    b2b = np.ascontiguousarray(np.tile(0.5 * np.concatenate([b2[0], b2[1]]), (P, 1)))

    in_maps = []
    for cc in range(NC):
        base = cc * SHARD
        idxA, colfA = build_stream(cc, 0, NBA)
        idxB, colfB = build_stream(cc, 1, NBB)
        dinvo = 0.5 * dinv_full[base:base + SHARD].reshape(c.W, P).T
        in_maps.append({
            "xT": xT,
            "xTow": np.ascontiguousarray(xpad[base:base + SHARD].T.astype(np.float16)),
            "w1cat": w1cat, "rw1c": rw1c, "w2cat": w2cat, "rw2c": rw2c,
            "b1b": b1b, "b2b": b2b,
            "dinvr": dinvr,
            "dinvo": np.ascontiguousarray(dinvo.astype(np.float32)),
            "idxA": idxA, "idxB": idxB,
            "colfA": colfA, "colfB": colfB,
        })
    return in_maps, NBA, NBB


# --------------------------------------------------------------------------
# device program
# --------------------------------------------------------------------------
def _build_program(c: Cfg, NBA, NBB):
    import concourse.tile as tile
    from concourse import bacc, mybir
    from concourse.masks import make_identity

    f32 = mybir.dt.float32
    f16 = mybir.dt.float16
    i16 = mybir.dt.int16
    AL = mybir.AluOpType
    AF = mybir.ActivationFunctionType

    NBAtot, NBBtot = sum(NBA), sum(NBB)
    LA, LB = NBAtot * P, NBBtot * P

    nc = bacc.Bacc("TRN2", target_bir_lowering=False, debug=False,
                   num_devices=c.NC, num_swdge_queues=4)
    qrr = [0]  # gather queue round-robin

    def din(name, shape, dt=f32):
        return nc.dram_tensor(name, shape, dt, kind="ExternalInput")

    xT = din("xT", [P, c.ROWS], f16)
    xTow = din("xTow", [P, c.SHARD], f16)
    w1cat = din("w1cat", [P, 128], f16); rw1c = din("rw1c", [P, 128], f16)
    w2cat = din("w2cat", [64, 128], f16); rw2c = din("rw2c", [64, 128], f16)
    b1b = din("b1b", [P, 128]); b2b = din("b2b", [P, 128])
    dinvr = din("dinvr", [P, c.NCHUNK])
    dinvo = din("dinvo", [P, c.W])
    idxA = din("idxA", [P, LA // 16], i16)
    idxB = din("idxB", [P, LB // 16], i16)
    colfA = din("colfA", [P, NBAtot], f16)
    colfB = din("colfB", [P, NBBtot], f16)
    yt = nc.dram_tensor("yt", [64, c.SHARD], f32, kind="ExternalOutput")

    t1A = nc.dram_tensor("t1A", [c.HALFA, 128], f16)
    t1B = nc.dram_tensor("t1B", [c.HALFB, 128], f16)
    t2A = nc.dram_tensor("t2A", [c.HALFA, 128], f16)
    t2B = nc.dram_tensor("t2B", [c.HALFB, 128], f16)
    ccin = nc.dram_tensor("ccin", [64, c.SHARD], f16)
    ccout = nc.dram_tensor("ccout", [c.NC, 64, c.SHARD], f16)

    # table2 chunk -> (shard, local node chunk); zero chunks -> None
    chunk_src = [None] * c.NCHUNK
    for s in range(c.NC):
        for j in range(c.W):
            node0 = s * c.SHARD + j * P
            r0 = node0 + (c.ZPAD if node0 >= c.B0 else 0)
            chunk_src[r0 // P] = (s, j)
    runs = []
    i = 0
    while i < c.NCHUNK:
        if chunk_src[i] is None:
            j = i
            while j < c.NCHUNK and chunk_src[j] is None:
                j += 1
            runs.append(("zero", i, j - i, 0))
            i = j
        else:
            s0, l0 = chunk_src[i]
            j = i
            while (j < c.NCHUNK and chunk_src[j] is not None
                   and chunk_src[j][0] == s0
                   and chunk_src[j][1] == l0 + (j - i) and (j - i) < 12):
                j += 1
            runs.append((s0, i, j - i, l0))
            i = j

    with tile.TileContext(nc) as tc, ExitStack() as ctx:
        cpool = ctx.enter_context(tc.tile_pool(name="consts", bufs=1))
        xtp = ctx.enter_context(tc.tile_pool(name="xtp", bufs=3))
        stg = ctx.enter_context(tc.tile_pool(name="stg", bufs=3))
        gth = ctx.enter_context(tc.tile_pool(name="gth", bufs=4))
        sgp = ctx.enter_context(tc.tile_pool(name="sgp", bufs=8))
        idxp = ctx.enter_context(tc.tile_pool(name="idxp", bufs=2))
        epi = ctx.enter_context(tc.tile_pool(name="epi", bufs=3))
        big = ctx.enter_context(tc.tile_pool(name="big", bufs=1))
        shp = ctx.enter_context(tc.tile_pool(name="shp", bufs=2))
        psx = ctx.enter_context(tc.tile_pool(name="psx", bufs=2, space="PSUM"))
        psw = ctx.enter_context(tc.tile_pool(name="psw", bufs=3, space="PSUM"))

        # ---- constants ----
        ident = cpool.tile([P, P], f32, tag="ident")
        make_identity(nc, ident[:])
        iota_i = cpool.tile([P, c.G * P], mybir.dt.int32, tag="iotai")
        nc.gpsimd.iota(iota_i[:], pattern=[[0, c.G], [1, P]], base=0,
                       channel_multiplier=0)
        iota_f = cpool.tile([P, c.G * P], f16, tag="iotaf")
        nc.vector.tensor_copy(iota_f[:], iota_i[:])
        zero64 = cpool.tile([64, 128], f16, tag="zero64")
        nc.vector.memset(zero64[:], 0.0)

        def load_const(dram, shape, tag, dt=f32):
            t = cpool.tile(shape, dt, tag=tag)
            nc.sync.dma_start(t[:], dram[:, :])
            return t

        w1_s = load_const(w1cat, [P, 128], "w1", f16)
        rw1_s = load_const(rw1c, [P, 128], "rw1", f16)
        w2_s = load_const(w2cat, [64, 128], "w2", f16)
        rw2_s = load_const(rw2c, [64, 128], "rw2", f16)
        b1_s = load_const(b1b, [P, 128], "b1")
        b2_s = load_const(b2b, [P, 128], "b2")
        dinvr_s = load_const(dinvr, [P, c.NCHUNK], "dinvr")
        dinvo_s = load_const(dinvo, [P, c.W], "dinvo")
        colfA_s = load_const(colfA, [P, NBAtot], "colfA", f16)
        colfB_s = load_const(colfB, [P, NBBtot], "colfB", f16)

        def half_base(t):
            return 0 if t in (t1A, t2A) else c.ACH

        # ---- staged table builder: chunk_list = [(tgt, rc, lhsT_ap)] ----
        def build_table(chunk_list, rhs_tile):
            i = 0
            while i < len(chunk_list):
                grp = chunk_list[i:i + 8]
                tgt = grp[0][0]
                grp = [g for g in grp if g[0] is tgt]
                st = stg.tile([P, 8 * 128], f16, tag="stage")
                for j, (_, rc, lap) in enumerate(grp):
                    ps = psx.tile([P, 128], f32)
                    nc.tensor.matmul(out=ps[:], lhsT=lap, rhs=rhs_tile[:],
                                     start=True, stop=True)
                    nc.vector.tensor_scalar(
                        out=st[:, j * 128:(j + 1) * 128], in0=ps[:],
                        scalar1=dinvr_s[:, rc:rc + 1], scalar2=None,
                        op0=AL.mult)
                r0 = grp[0][1] - half_base(tgt)
                n = len(grp)
                out_ap = tgt[r0 * P:(r0 + n) * P, :] \
                    .rearrange("(k p) f -> p k f", p=P)
                nc.sync.dma_start(out_ap, st[:, :n * 128])
                i += n

        # ---- layer-1 tables (A half then B half) + root1 ----
        with nc.named_scope("prolog"):
            i = 0
            chunk_list1 = []
            while i < c.NCHUNK:
                n = min(8, c.NCHUNK - i)
                if i < c.ACH < i + n:
                    n = c.ACH - i
                xp = xtp.tile([P, 8 * 128], f16, tag="xtp")
                nc.sync.dma_start(xp[:, :n * 128], xT[:, i * P:(i + n) * P])
                for j in range(n):
                    rc = i + j
                    tgt = t1A if rc < c.ACH else t1B
                    chunk_list1.append((tgt, rc, xp[:, j * 128:(j + 1) * 128]))
                i += n
            build_table(chunk_list1, w1_s)

            root1 = big.tile([P, c.SHARD], f32, tag="root")
            i = 0
            while i < c.W:
                n = min(8, c.W - i)
                xp = xtp.tile([P, 8 * 128], f16, tag="xtp")
                nc.sync.dma_start(xp[:, :n * 128], xTow[:, i * P:(i + n) * P])
                for j in range(n):
                    ps = psx.tile([P, 128], f32)
                    nc.tensor.matmul(out=ps[:], lhsT=xp[:, j * 128:(j + 1) * 128],
                                     rhs=rw1_s[:], start=True, stop=True)
                    nc.vector.tensor_tensor(
                        out=root1[:, (i + j) * 128:(i + j + 1) * 128],
                        in0=ps[:], in1=b1_s[:], op=AL.add)
                i += n

        # ---- gather/aggregate layer ----
        def layer(tabA, tabB, root_t, out_t, out_dt):
            blkA = blkB = 0
            tokA = tokB = 0
            for sb in c.sbatches:
                sbA = sum(NBA[w] for b in sb for w in b) * P
                sbB = sum(NBB[w] for b in sb for w in b) * P
                ixA = idxp.tile([P, sbA // 16], i16, tag="ixA")
                nc.sync.dma_start(ixA[:], idxA[:, tokA // 16:(tokA + sbA) // 16])
                ixB = idxp.tile([P, sbB // 16], i16, tag="ixB")
                nc.sync.dma_start(ixB[:], idxB[:, tokB // 16:(tokB + sbB) // 16])
                lA = lB = 0
                for batch in sb:
                    nA = sum(NBA[w] for w in batch)
                    nB = sum(NBB[w] for w in batch)
                    gA = gth.tile([P, nA * 128], f16, tag="gath")
                    nc.gpsimd.dma_gather(
                        out_ap=gA[:].rearrange("p (b f) -> p b f", f=128),
                        in_ap=tabA[:, :],
                        idxs_ap=ixA[:, lA // 16:(lA + nA * P) // 16],
                        num_idxs=nA * P, num_idxs_reg=nA * P, elem_size=128,
                        single_packet=False, queue_num=qrr[0] % 4)
                    qrr[0] += 1
                    gB = gth.tile([P, nB * 128], f16, tag="gath")
                    nc.gpsimd.dma_gather(
                        out_ap=gB[:].rearrange("p (b f) -> p b f", f=128),
                        in_ap=tabB[:, :],
                        idxs_ap=ixB[:, lB // 16:(lB + nB * P) // 16],
                        num_idxs=nB * P, num_idxs_reg=nB * P, elem_size=128,
                        single_packet=False, queue_num=qrr[0] % 4)
                    qrr[0] += 1
                    sA, sB_ = [], []
                    for g0 in range(0, nA, c.G):
                        gl = min(c.G, nA - g0)
                        s_t = sgp.tile([P, c.G * 128], f16, tag="sg")
                        nc.vector.tensor_tensor(
                            out=s_t[:, :gl * 128], in0=iota_f[:, :gl * 128],
                            in1=colfA_s[:, blkA + g0:blkA + g0 + gl]
                                .to_broadcast([P, gl, 128]),
                            op=AL.is_equal)
                        sA.append(s_t)
                    for g0 in range(0, nB, c.G):
                        gl = min(c.G, nB - g0)
                        s_t = sgp.tile([P, c.G * 128], f16, tag="sg")
                        nc.vector.tensor_tensor(
                            out=s_t[:, :gl * 128], in0=iota_f[:, :gl * 128],
                            in1=colfB_s[:, blkB + g0:blkB + g0 + gl]
                                .to_broadcast([P, gl, 128]),
                            op=AL.is_equal)
                        sB_.append(s_t)
                    oA = oB = 0
                    for w in batch:
                        pw = psw.tile([P, 128], f32)
                        nmm = NBA[w] + NBB[w]
                        k = 0
                        for j in range(NBA[w]):
                            b = oA + j
                            nc.tensor.matmul(
                                out=pw[:],
                                lhsT=sA[b // c.G][:, (b % c.G) * 128:(b % c.G + 1) * 128],
                                rhs=gA[:, b * 128:(b + 1) * 128],
                                start=(k == 0), stop=(k == nmm - 1))
                            k += 1
                        for j in range(NBB[w]):
                            b = oB + j
                            nc.tensor.matmul(
                                out=pw[:],
                                lhsT=sB_[b // c.G][:, (b % c.G) * 128:(b % c.G + 1) * 128],
                                rhs=gB[:, b * 128:(b + 1) * 128],
                                start=(k == 0), stop=(k == nmm - 1))
                            k += 1
                        oA += NBA[w]; oB += NBB[w]
                        t2 = epi.tile([P, 128], f32, tag="t2")
                        nc.vector.scalar_tensor_tensor(
                            out=t2[:], in0=pw[:], scalar=dinvo_s[:, w:w + 1],
                            in1=root_t[:, w * 128:(w + 1) * 128],
                            op0=AL.mult, op1=AL.add)
                        t3 = epi.tile([P, 128], f32, tag="t3")
                        nc.scalar.activation(t3[:], t2[:], AF.Relu)
                        t4 = epi.tile([P, 64], f32, tag="t4")
                        nc.vector.tensor_tensor(out=t4[:], in0=t3[:, :64],
                                                in1=t3[:, 64:], op=AL.add)
                        pt = psx.tile([64, 128], f32)
                        nc.tensor.transpose(out=pt[:], in_=t4[:],
                                            identity=ident[:])
                        nc.scalar.copy(out_t[:, w * 128:(w + 1) * 128], pt[:])
                    blkA += nA; blkB += nB
                    lA += nA * P; lB += nB * P
                tokA += sbA; tokB += sbB

        h1t = big.tile([64, c.SHARD], f16, tag="ht")
        with nc.named_scope("layer1"):
            layer(t1A, t1B, root1, h1t, f16)

        with nc.named_scope("cc"):
            nc.sync.dma_start(ccin[:, :], h1t[:])
            nc.gpsimd.collective_compute(
                "AllGather", AL.bypass,
                replica_groups=[list(range(c.NC))],
                ins=[ccin.ap().opt()], outs=[ccout.ap().opt()])

        with nc.named_scope("mid"):
            root2 = big.tile([P, c.SHARD], f32, tag="root")
            for j in range(c.W):
                ps = psx.tile([P, 128], f32)
                nc.tensor.matmul(out=ps[:], lhsT=h1t[:, j * 128:(j + 1) * 128],
                                 rhs=rw2_s[:], start=True, stop=True)
                nc.vector.tensor_tensor(out=root2[:, j * 128:(j + 1) * 128],
                                        in0=ps[:], in1=b2_s[:], op=AL.add)

            chunk_list2 = []
            for run in runs:
                kind, rc0, n, l0 = run
                if kind == "zero":
                    for j in range(n):
                        rc = rc0 + j
                        tgt = t2A if rc < c.ACH else t2B
                        chunk_list2.append((tgt, rc, zero64[:]))
                else:
                    pc = shp.tile([64, 12 * 128], f16, tag="h1pc")
                    nc.sync.dma_start(pc[:, :n * 128],
                                      ccout[kind, :, l0 * P:(l0 + n) * P])
                    for j in range(n):
                        rc = rc0 + j
                        tgt = t2A if rc < c.ACH else t2B
                        chunk_list2.append((tgt, rc, pc[:, j * 128:(j + 1) * 128]))
            build_table(chunk_list2, w2_s)

        yts = big.tile([64, c.SHARD], f32, tag="ht")
        with nc.named_scope("layer2"):
            layer(t2A, t2B, root2, yts, f32)
        nc.sync.dma_start(yt[:, :], yts[:])

    nc.compile()
    return nc


# --------------------------------------------------------------------------
# entry points
# --------------------------------------------------------------------------
_cache = {}


def prepare(inputs, cfg_kw=None):
    c = Cfg(**(cfg_kw or REAL))
    in_maps, NBA, NBB = _preprocess(c, **inputs)
    key = (tuple(sorted((cfg_kw or REAL).items())), tuple(NBA), tuple(NBB))
    if key not in _cache:
        _cache[key] = _build_program(c, NBA, NBB)
    return c, _cache[key], in_maps


def kernel(x, edge_index, init_w1, root_w1, b1, init_w2, root_w2, b2,
           _trace=False, _cfg=None):
    from concourse import bass_utils
    inputs = dict(x=np.asarray(x), edge_index=np.asarray(edge_index),
                  init_w1=np.asarray(init_w1), root_w1=np.asarray(root_w1),
                  b1=np.asarray(b1), init_w2=np.asarray(init_w2),
                  root_w2=np.asarray(root_w2), b2=np.asarray(b2))
    c, nc, in_maps = prepare(inputs, _cfg)
    res = bass_utils.run_bass_kernel_spmd(
        nc, in_maps, core_ids=list(range(c.NC)), trace=_trace)
    out = np.concatenate([res.results[cc]["yt"].T for cc in range(c.NC)],
                         axis=0)[:c.N]
    if _trace:
        kernel._last = res
    return np.ascontiguousarray(out.astype(np.float32))


# revision 6
# speedup vs baseline: 2.3490x; 1.0906x over previous
"""ARMA GNN (2-layer, K=2 stacks) Trainium2 kernel.

Strategy (8-core SPMD, node-sharded):
  * norm folding: norm[e] = dinv[row]*dinv[col].  Gather tables hold
    dinv[row]*x[row] @ Wcat (both stacks concatenated on the feature axis,
    fp16), dinv[col] is applied per target window at the epilogue.
  * Edges are sharded by target (col) across cores; each core owns
    SHARD consecutive nodes = W windows of 128 target slots.
  * Per 128-edge block: gather source rows with dma_gather (SWDGE custom
    DMA, 4 queues round-robin so all four Q7 core pairs generate
    descriptors concurrently), build a 0/1 selection matrix
    S[e, slot] = (iota == slot[e]) on DVE (fp16), and matmul S.T @ M on
    PE (fp16 with FWL), accumulating windows in f32 PSUM.
  * Sources are split in two table halves (row index must fit int16).
  * Between layers: one AllGather of the [64, SHARD] fp16 h1^T shards.
  * All programs identical across cores (block counts padded to the
    max over cores per window); per-core behavior comes from input data.

kernel(**inputs) takes the FULL problem inputs and returns the FULL output.
"""

import sys

sys.path.insert(0, "/opt/trn_rl_repo")

from contextlib import ExitStack

import numpy as np

P = 128


# --------------------------------------------------------------------------
# configuration
# --------------------------------------------------------------------------
class Cfg:
    def __init__(self, N, NC, SHARD, B0, ZPAD=256, WB=2, G=8, SUPER=4,
                 FIN=128, HID=64, FOUT=64, K=2):
        self.N, self.NC, self.SHARD, self.B0, self.ZPAD = N, NC, SHARD, B0, ZPAD
        self.WB, self.G, self.SUPER = WB, G, SUPER
        self.FIN, self.HID, self.FOUT, self.K = FIN, HID, FOUT, K
        self.NSTAR = NC * SHARD
        self.W = SHARD // P
        self.ROWS = self.NSTAR + ZPAD
        self.HALFA = B0 + ZPAD
        self.HALFB = self.NSTAR - B0
        self.NCHUNK = self.ROWS // P
        self.ACH = self.HALFA // P      # chunks in half A
        self.BCH = self.HALFB // P
        assert B0 % P == 0 and SHARD % P == 0 and ZPAD % P == 0
        assert self.HALFA < 32768 and self.HALFB < 32768
        assert N > B0 and N < self.NSTAR
        assert K * HID == 128 and K * FOUT == 128 and FIN == 128
        self.ZLOCA = B0                       # zero row (local) in half A
        self.ZLOCB = (N + ZPAD) - self.HALFA  # zero row (local) in half B
        self.batches = [tuple(range(b, min(b + WB, self.W)))
                        for b in range(0, self.W, WB)]
        self.sbatches = [self.batches[i:i + SUPER]
                         for i in range(0, len(self.batches), SUPER)]


REAL = dict(N=50000, NC=8, SHARD=6272, B0=24960)


# --------------------------------------------------------------------------
# host preprocessing
# --------------------------------------------------------------------------
def _preprocess(c: Cfg, x, edge_index, init_w1, root_w1, b1, init_w2, root_w2, b2):
    N, NC, SHARD = c.N, c.NC, c.SHARD
    row = np.asarray(edge_index[0]).astype(np.int64)
    col = np.asarray(edge_index[1]).astype(np.int64)
    x = np.asarray(x, dtype=np.float32)

    deg = np.bincount(col, minlength=N).astype(np.float64)
    dinv = np.where(deg > 0, deg ** -0.5, 0.0).astype(np.float32)
    dinv_full = np.zeros(c.NSTAR, np.float32)
    dinv_full[:N] = dinv

    srow = row + (row >= c.B0) * c.ZPAD  # table row of each source

    percore = []
    counts = np.zeros((NC, c.W, 2), np.int64)
    for cc in range(NC):
        base = cc * SHARD
        m = (col >= base) & (col < base + SHARD)
        ec = (col[m] - base).astype(np.int64)
        es = srow[m]
        half = (es >= c.HALFA).astype(np.int64)
        key = (ec >> 7) * 2 + half
        order = np.argsort(key, kind="stable")
        ec, es, key = ec[order], es[order], key[order]
        bounds = np.searchsorted(key, np.arange(2 * c.W + 1))
        percore.append((ec, es, bounds))
        for w in range(c.W):
            counts[cc, w, 0] = bounds[2 * w + 1] - bounds[2 * w]
            counts[cc, w, 1] = bounds[2 * w + 2] - bounds[2 * w + 1]

    NBA = [max(1, int(-(-counts[:, w, 0].max() // P))) for w in range(c.W)]
    NBB = [max(1, int(-(-counts[:, w, 1].max() // P))) for w in range(c.W)]

    def build_stream(cc, half_id, NB_list):
        ec, es, bounds = percore[cc]
        toks, slots = [], []
        zloc = c.ZLOCA if half_id == 0 else c.ZLOCB
        for batch in c.batches:
            for w in batch:
                lo, hi = bounds[2 * w + half_id], bounds[2 * w + half_id + 1]
                k = hi - lo
                n = NB_list[w] * P
                t = np.full(n, zloc, np.int64)
                s = np.zeros(n, np.int64)
                t[:k] = es[lo:hi] - (c.HALFA if half_id else 0)
                s[:k] = ec[lo:hi] & 127
                toks.append(t)
                slots.append(s)
        toks = np.concatenate(toks)
        slots = np.concatenate(slots)
        L = len(toks)
        idxw = np.tile(toks.reshape(L // 16, 16).T.astype(np.int16), (8, 1))
        colf = np.ascontiguousarray(slots.reshape(L // P, P).T.astype(np.float16))
        return np.ascontiguousarray(idxw), colf

    xrow = np.zeros((c.ROWS, c.FIN), np.float32)
    xrow[:c.B0] = x[:c.B0]
    xrow[c.HALFA:c.HALFA + (N - c.B0)] = x[c.B0:N]
    xT = np.ascontiguousarray(xrow.T.astype(np.float16))

    dinvrow = np.zeros(c.ROWS, np.float32)
    dinvrow[:c.B0] = dinv_full[:c.B0]
    dinvrow[c.HALFA:] = dinv_full[c.B0:]
    dinvr = np.ascontiguousarray(dinvrow.reshape(c.NCHUNK, P).T)

    xpad = np.zeros((c.NSTAR, c.FIN), np.float32)
    xpad[:N] = x

    def cat2(w):  # [K, fi, fo] -> [fi, K*fo] fp16
        w = np.asarray(w, dtype=np.float32)
        return np.ascontiguousarray(
            np.concatenate([w[0], w[1]], axis=1).astype(np.float16))

    w1cat = cat2(init_w1)
    rw1c = cat2(0.5 * np.asarray(root_w1, np.float32))
    w2cat = cat2(init_w2)
    rw2c = cat2(0.5 * np.asarray(root_w2, np.float32))
    b1 = np.asarray(b1, dtype=np.float32)
    b2 = np.asarray(b2, dtype=np.float32)
    b1b = np.ascontiguousarray(np.tile(0.5 * np.concatenate([b1[0], b1[1]]), (P, 1)))
    b2b = np.ascontiguousarray(np.tile(0.5 * np.concatenate([b2[0], b2[1]]), (P, 1)))

    in_maps = []
    for cc in range(NC):
        base = cc * SHARD
        idxA, colfA = build_stream(cc, 0, NBA)
        idxB, colfB = build_stream(cc, 1, NBB)
        dinvo = 0.5 * dinv_full[base:base + SHARD].reshape(c.W, P).T
        in_maps.append({
            "xT": xT,
            "xTow": np.ascontiguousarray(xpad[base:base + SHARD].T.astype(np.float16)),
            "w1cat": w1cat, "rw1c": rw1c, "w2cat": w2cat, "rw2c": rw2c,
            "b1b": b1b, "b2b": b2b,
            "dinvr": dinvr,
            "dinvo": np.ascontiguousarray(dinvo.astype(np.float32)),
            "idxA": idxA, "idxB": idxB,
            "colfA": colfA, "colfB": colfB,
        })
    return in_maps, NBA, NBB


# --------------------------------------------------------------------------
# device program
# --------------------------------------------------------------------------
def _build_program(c: Cfg, NBA, NBB):
    import concourse.tile as tile
    from concourse import bacc, mybir
    from concourse.masks import make_identity

    f32 = mybir.dt.float32
    f16 = mybir.dt.float16
    i16 = mybir.dt.int16
    AL = mybir.AluOpType
    AF = mybir.ActivationFunctionType

    NBAtot, NBBtot = sum(NBA), sum(NBB)
    LA, LB = NBAtot * P, NBBtot * P

    nc = bacc.Bacc("TRN2", target_bir_lowering=False, debug=False,
                   num_devices=c.NC, num_swdge_queues=4)
    qrr = [0]  # gather queue round-robin

    def din(name, shape, dt=f32):
        return nc.dram_tensor(name, shape, dt, kind="ExternalInput")

    xT = din("xT", [P, c.ROWS], f16)
    xTow = din("xTow", [P, c.SHARD], f16)
    w1cat = din("w1cat", [P, 128], f16); rw1c = din("rw1c", [P, 128], f16)
    w2cat = din("w2cat", [64, 128], f16); rw2c = din("rw2c", [64, 128], f16)
    b1b = din("b1b", [P, 128]); b2b = din("b2b", [P, 128])
    dinvr = din("dinvr", [P, c.NCHUNK])
    dinvo = din("dinvo", [P, c.W])
    idxA = din("idxA", [P, LA // 16], i16)
    idxB = din("idxB", [P, LB // 16], i16)
    colfA = din("colfA", [P, NBAtot], f16)
    colfB = din("colfB", [P, NBBtot], f16)
    yt = nc.dram_tensor("yt", [64, c.SHARD], f32, kind="ExternalOutput")

    t1A = nc.dram_tensor("t1A", [c.HALFA, 128], f16)
    t1B = nc.dram_tensor("t1B", [c.HALFB, 128], f16)
    t2A = nc.dram_tensor("t2A", [c.HALFA, 128], f16)
    t2B = nc.dram_tensor("t2B", [c.HALFB, 128], f16)
    ccin = nc.dram_tensor("ccin", [64, c.SHARD], f16)
    ccout = nc.dram_tensor("ccout", [c.NC, 64, c.SHARD], f16)

    # table2 chunk -> (shard, local node chunk); zero chunks -> None
    chunk_src = [None] * c.NCHUNK
    for s in range(c.NC):
        for j in range(c.W):
            node0 = s * c.SHARD + j * P
            r0 = node0 + (c.ZPAD if node0 >= c.B0 else 0)
            chunk_src[r0 // P] = (s, j)
    runs = []
    i = 0
    while i < c.NCHUNK:
        if chunk_src[i] is None:
            j = i
            while j < c.NCHUNK and chunk_src[j] is None:
                j += 1
            runs.append(("zero", i, j - i, 0))
            i = j
        else:
            s0, l0 = chunk_src[i]
            j = i
            while (j < c.NCHUNK and chunk_src[j] is not None
                   and chunk_src[j][0] == s0
                   and chunk_src[j][1] == l0 + (j - i) and (j - i) < 12):
                j += 1
            runs.append((s0, i, j - i, l0))
            i = j

    with tile.TileContext(nc) as tc, ExitStack() as ctx:
        cpool = ctx.enter_context(tc.tile_pool(name="consts", bufs=1))
        xtp = ctx.enter_context(tc.tile_pool(name="xtp", bufs=3))
        stg = ctx.enter_context(tc.tile_pool(name="stg", bufs=3))
        gth = ctx.enter_context(tc.tile_pool(name="gth", bufs=6))
        sgp = ctx.enter_context(tc.tile_pool(name="sgp", bufs=8))
        idxp = ctx.enter_context(tc.tile_pool(name="idxp", bufs=3))
        epi = ctx.enter_context(tc.tile_pool(name="epi", bufs=3))
        big = ctx.enter_context(tc.tile_pool(name="big", bufs=1))
        shp = ctx.enter_context(tc.tile_pool(name="shp", bufs=2))
        psx = ctx.enter_context(tc.tile_pool(name="psx", bufs=2, space="PSUM"))
        psw = ctx.enter_context(tc.tile_pool(name="psw", bufs=3, space="PSUM"))

        # ---- constants ----
        ident = cpool.tile([P, P], f32, tag="ident")
        make_identity(nc, ident[:])
        iota_i = cpool.tile([P, c.G * P], mybir.dt.int32, tag="iotai")
        nc.gpsimd.iota(iota_i[:], pattern=[[0, c.G], [1, P]], base=0,
                       channel_multiplier=0)
        iota_f = cpool.tile([P, c.G * P], f16, tag="iotaf")
        nc.vector.tensor_copy(iota_f[:], iota_i[:])
        zero64 = cpool.tile([64, 128], f16, tag="zero64")
        nc.vector.memset(zero64[:], 0.0)

        def load_const(dram, shape, tag, dt=f32):
            t = cpool.tile(shape, dt, tag=tag)
            nc.sync.dma_start(t[:], dram[:, :])
            return t

        w1_s = load_const(w1cat, [P, 128], "w1", f16)
        rw1_s = load_const(rw1c, [P, 128], "rw1", f16)
        w2_s = load_const(w2cat, [64, 128], "w2", f16)
        rw2_s = load_const(rw2c, [64, 128], "rw2", f16)
        b1_s = load_const(b1b, [P, 128], "b1")
        b2_s = load_const(b2b, [P, 128], "b2")
        dinvr_s = load_const(dinvr, [P, c.NCHUNK], "dinvr")
        dinvo_s = load_const(dinvo, [P, c.W], "dinvo")
        colfA_s = load_const(colfA, [P, NBAtot], "colfA", f16)
        colfB_s = load_const(colfB, [P, NBBtot], "colfB", f16)

        def half_base(t):
            return 0 if t in (t1A, t2A) else c.ACH

        # ---- staged table builder: chunk_list = [(tgt, rc, lhsT_ap)] ----
        def build_table(chunk_list, rhs_tile):
            i = 0
            while i < len(chunk_list):
                grp = chunk_list[i:i + 8]
                tgt = grp[0][0]
                grp = [g for g in grp if g[0] is tgt]
                st = stg.tile([P, 8 * 128], f16, tag="stage")
                for j, (_, rc, lap) in enumerate(grp):
                    ps = psx.tile([P, 128], f32)
                    nc.tensor.matmul(out=ps[:], lhsT=lap, rhs=rhs_tile[:],
                                     start=True, stop=True)
                    nc.scalar.mul(st[:, j * 128:(j + 1) * 128], ps[:],
                                  dinvr_s[:, rc:rc + 1])
                r0 = grp[0][1] - half_base(tgt)
                n = len(grp)
                out_ap = tgt[r0 * P:(r0 + n) * P, :] \
                    .rearrange("(k p) f -> p k f", p=P)
                nc.sync.dma_start(out_ap, st[:, :n * 128])
                i += n

        # ---- layer-1 tables (A half then B half) + root1 ----
        with nc.named_scope("prolog"):
            i = 0
            chunk_list1 = []
            while i < c.NCHUNK:
                n = min(8, c.NCHUNK - i)
                if i < c.ACH < i + n:
                    n = c.ACH - i
                xp = xtp.tile([P, 8 * 128], f16, tag="xtp")
                nc.sync.dma_start(xp[:, :n * 128], xT[:, i * P:(i + n) * P])
                for j in range(n):
                    rc = i + j
                    tgt = t1A if rc < c.ACH else t1B
                    chunk_list1.append((tgt, rc, xp[:, j * 128:(j + 1) * 128]))
                i += n
            build_table(chunk_list1, w1_s)

            root1 = big.tile([P, c.SHARD], f32, tag="root")
            i = 0
            while i < c.W:
                n = min(8, c.W - i)
                xp = xtp.tile([P, 8 * 128], f16, tag="xtp")
                nc.sync.dma_start(xp[:, :n * 128], xTow[:, i * P:(i + n) * P])
                for j in range(n):
                    ps = psx.tile([P, 128], f32)
                    nc.tensor.matmul(out=ps[:], lhsT=xp[:, j * 128:(j + 1) * 128],
                                     rhs=rw1_s[:], start=True, stop=True)
                    nc.vector.tensor_tensor(
                        out=root1[:, (i + j) * 128:(i + j + 1) * 128],
                        in0=ps[:], in1=b1_s[:], op=AL.add)
                i += n

        # ---- gather/aggregate layer ----
        def layer(tabA, tabB, root_t, out_t, out_dt):
            blkA = blkB = 0
            tokA = tokB = 0
            for sb in c.sbatches:
                sbA = sum(NBA[w] for b in sb for w in b) * P
                sbB = sum(NBB[w] for b in sb for w in b) * P
                ixA = idxp.tile([P, sbA // 16], i16, tag="ixA")
                nc.sync.dma_start(ixA[:], idxA[:, tokA // 16:(tokA + sbA) // 16])
                ixB = idxp.tile([P, sbB // 16], i16, tag="ixB")
                nc.sync.dma_start(ixB[:], idxB[:, tokB // 16:(tokB + sbB) // 16])
                lA = lB = 0
                for batch in sb:
                    nA = sum(NBA[w] for w in batch)
                    nB = sum(NBB[w] for w in batch)
                    gA = gth.tile([P, nA * 128], f16, tag="gath")
                    nc.gpsimd.dma_gather(
                        out_ap=gA[:].rearrange("p (b f) -> p b f", f=128),
                        in_ap=tabA[:, :],
                        idxs_ap=ixA[:, lA // 16:(lA + nA * P) // 16],
                        num_idxs=nA * P, num_idxs_reg=nA * P, elem_size=128,
                        single_packet=False, queue_num=qrr[0] % 4)
                    qrr[0] += 1
                    gB = gth.tile([P, nB * 128], f16, tag="gath")
                    nc.gpsimd.dma_gather(
                        out_ap=gB[:].rearrange("p (b f) -> p b f", f=128),
                        in_ap=tabB[:, :],
                        idxs_ap=ixB[:, lB // 16:(lB + nB * P) // 16],
                        num_idxs=nB * P, num_idxs_reg=nB * P, elem_size=128,
                        single_packet=False, queue_num=qrr[0] % 4)
                    qrr[0] += 1
                    sA, sB_ = [], []
                    for g0 in range(0, nA, c.G):
                        gl = min(c.G, nA - g0)
                        s_t = sgp.tile([P, c.G * 128], f16, tag="sg")
                        nc.vector.tensor_tensor(
                            out=s_t[:, :gl * 128], in0=iota_f[:, :gl * 128],
                            in1=colfA_s[:, blkA + g0:blkA + g0 + gl]
                                .to_broadcast([P, gl, 128]),
                            op=AL.is_equal)
                        sA.append(s_t)
                    for g0 in range(0, nB, c.G):
                        gl = min(c.G, nB - g0)
                        s_t = sgp.tile([P, c.G * 128], f16, tag="sg")
                        nc.vector.tensor_tensor(
                            out=s_t[:, :gl * 128], in0=iota_f[:, :gl * 128],
                            in1=colfB_s[:, blkB + g0:blkB + g0 + gl]
                                .to_broadcast([P, gl, 128]),
                            op=AL.is_equal)
                        sB_.append(s_t)
                    oA = oB = 0
                    for w in batch:
                        pw = psw.tile([P, 128], f32)
                        nmm = NBA[w] + NBB[w]
                        k = 0
                        for j in range(NBA[w]):
                            b = oA + j
                            nc.tensor.matmul(
                                out=pw[:],
                                lhsT=sA[b // c.G][:, (b % c.G) * 128:(b % c.G + 1) * 128],
                                rhs=gA[:, b * 128:(b + 1) * 128],
                                start=(k == 0), stop=(k == nmm - 1))
                            k += 1
                        for j in range(NBB[w]):
                            b = oB + j
                            nc.tensor.matmul(
                                out=pw[:],
                                lhsT=sB_[b // c.G][:, (b % c.G) * 128:(b % c.G + 1) * 128],
                                rhs=gB[:, b * 128:(b + 1) * 128],
                                start=(k == 0), stop=(k == nmm - 1))
                            k += 1
                        oA += NBA[w]; oB += NBB[w]
                        t2 = epi.tile([P, 128], f32, tag="t2")
                        nc.vector.scalar_tensor_tensor(
                            out=t2[:], in0=pw[:], scalar=dinvo_s[:, w:w + 1],
                            in1=root_t[:, w * 128:(w + 1) * 128],
                            op0=AL.mult, op1=AL.add)
                        t3 = epi.tile([P, 128], f32, tag="t3")
                        nc.scalar.activation(t3[:], t2[:], AF.Relu)
                        t4 = epi.tile([P, 64], f32, tag="t4")
                        nc.vector.tensor_tensor(out=t4[:], in0=t3[:, :64],
                                                in1=t3[:, 64:], op=AL.add)
                        pt = psx.tile([64, 128], f32)
                        nc.tensor.transpose(out=pt[:], in_=t4[:],
                                            identity=ident[:])
                        nc.scalar.copy(out_t[:, w * 128:(w + 1) * 128], pt[:])
                    blkA += nA; blkB += nB
                    lA += nA * P; lB += nB * P
                tokA += sbA; tokB += sbB

        h1t = big.tile([64, c.SHARD], f16, tag="ht")
        with nc.named_scope("layer1"):
            layer(t1A, t1B, root1, h1t, f16)

        with nc.named_scope("cc"):
            nc.sync.dma_start(ccin[:, :], h1t[:])
            nc.gpsimd.collective_compute(
                "AllGather", AL.bypass,
                replica_groups=[list(range(c.NC))],
                ins=[ccin.ap().opt()], outs=[ccout.ap().opt()])

        with nc.named_scope("mid"):
            chunk_list2 = []
            for run in runs:
                kind, rc0, n, l0 = run
                if kind == "zero":
                    for j in range(n):
                        rc = rc0 + j
                        tgt = t2A if rc < c.ACH else t2B
                        chunk_list2.append((tgt, rc, zero64[:]))
                else:
                    pc = shp.tile([64, 12 * 128], f16, tag="h1pc")
                    nc.sync.dma_start(pc[:, :n * 128],
                                      ccout[kind, :, l0 * P:(l0 + n) * P])
                    for j in range(n):
                        rc = rc0 + j
                        tgt = t2A if rc < c.ACH else t2B
                        chunk_list2.append((tgt, rc, pc[:, j * 128:(j + 1) * 128]))
            build_table(chunk_list2, w2_s)

            root2 = big.tile([P, c.SHARD], f32, tag="root")
            for j in range(c.W):
                ps = psx.tile([P, 128], f32)
                nc.tensor.matmul(out=ps[:], lhsT=h1t[:, j * 128:(j + 1) * 128],
                                 rhs=rw2_s[:], start=True, stop=True)
                nc.vector.tensor_tensor(out=root2[:, j * 128:(j + 1) * 128],
                                        in0=ps[:], in1=b2_s[:], op=AL.add)

        yts = big.tile([64, c.SHARD], f32, tag="ht")
        with nc.named_scope("layer2"):
            layer(t2A, t2B, root2, yts, f32)
        nc.sync.dma_start(yt[:, :], yts[:])

    nc.compile()
    return nc


# --------------------------------------------------------------------------
# entry points
# --------------------------------------------------------------------------
_cache = {}


def prepare(inputs, cfg_kw=None):
    c = Cfg(**(cfg_kw or REAL))
    in_maps, NBA, NBB = _preprocess(c, **inputs)
    key = (tuple(sorted((cfg_kw or REAL).items())), tuple(NBA), tuple(NBB))
    if key not in _cache:
        _cache[key] = _build_program(c, NBA, NBB)
    return c, _cache[key], in_maps


def kernel(x, edge_index, init_w1, root_w1, b1, init_w2, root_w2, b2,
           _trace=False, _cfg=None):
    from concourse import bass_utils
    inputs = dict(x=np.asarray(x), edge_index=np.asarray(edge_index),
                  init_w1=np.asarray(init_w1), root_w1=np.asarray(root_w1),
                  b1=np.asarray(b1), init_w2=np.asarray(init_w2),
                  root_w2=np.asarray(root_w2), b2=np.asarray(b2))
    c, nc, in_maps = prepare(inputs, _cfg)
    res = bass_utils.run_bass_kernel_spmd(
        nc, in_maps, core_ids=list(range(c.NC)), trace=_trace)
    out = np.concatenate([res.results[cc]["yt"].T for cc in range(c.NC)],
                         axis=0)[:c.N]
    if _trace:
        kernel._last = res
    return np.ascontiguousarray(out.astype(np.float32))
